# revision 25
# baseline (speedup 1.0000x reference)
"""Trainium2 Bass kernel: MultiHeadAttention with QK-RMSNorm + partial rotary,
causal softmax. B=4, T=2048, D=1024, H=16, HD=64, fp32 reference.

Sharding: 8 cores = 4 batches x 2 head-groups (8 heads each). Each core:
  - QKV projections for its batch, restricted to its 512 head-dims
  - causal attention for its 8 heads
  - partial output projection (its 512 contraction dims, all 1024 outputs)
Host sums the two head-group partials per batch (the all-reduce) and
transposes back.

v4 design (from v3): all-fp16 matmul data + wide-op rotary.
  - fp16 everywhere on the MM path (x, w, q, k, v, wo, rotary tables):
    same PE rate as bf16/f32r warm (1 cyc/row), no f32r cold 4x penalty,
    FWL on weight loads, half DMA bytes, 8-bit-mantissa accuracy.
  - q/k live in single [128, 4, T] tiles so the rotary/RMS-norm chain runs
    as wide [128,4,512] DVE ops (one per step instead of per-head-pair),
    cutting DVE busy in the rotary window ~2x; 16-bit DVE ops run 2x.
  - PSUM->SBUF bridge: xs*s2 writes an fp16 SBUF tile so the following
    add/mul stay 16-bit SBUF-only.
  - epilogue: reciprocal reads the denominator row straight from PSUM.
  - diagonal score strips computed at their true width (min N=128).
  - q/k projection copies on DVE, v copies on ACT (engine balance).
"""

import numpy as np
from contextlib import ExitStack

import concourse.bass as bass
import concourse.tile as tile
import concourse.mybir as mybir
from concourse import bacc

F32 = mybir.dt.float32
F16 = mybir.dt.float16
AF = mybir.ActivationFunctionType
MUL = mybir.AluOpType.mult
ADD = mybir.AluOpType.add

D = 1024   # model dim
DH = 512   # head-group width per core (8 heads x 64)
NH = 8     # heads per core
HD = 64    # head dim
NKC = D // 128   # k-chunks over model dim
EPS = 1e-6


def build_kernel(nc: bass.Bass, T: int = 2048):
    NTT = T // 512     # 512-wide t/i blocks
    NTS = T // 128     # 128-wide t/j chunks

    xt = nc.dram_tensor("xt", [D, T], F16, kind="ExternalInput").ap()
    wqt = nc.dram_tensor("wqt", [D, DH], F16, kind="ExternalInput").ap()
    wkt = nc.dram_tensor("wkt", [D, DH], F16, kind="ExternalInput").ap()
    wvt = nc.dram_tensor("wvt", [D, DH], F16, kind="ExternalInput").ap()
    wot = nc.dram_tensor("wot", [DH, D], F16, kind="ExternalInput").ap()
    c2d = nc.dram_tensor("c2", [128, 4, T], F16, kind="ExternalInput").ap()
    s2d = nc.dram_tensor("s2", [128, 4, T], F32, kind="ExternalInput").ap()
    pswapd = nc.dram_tensor("pswap", [128, 128], F16, kind="ExternalInput").ap()
    bdiagd = nc.dram_tensor("bdiag", [128, 128], F16, kind="ExternalInput").ap()
    trid = nc.dram_tensor("trimask", [128, 256], F16, kind="ExternalInput").ap()
    yt = nc.dram_tensor("yt", [D, T], F16, kind="ExternalOutput").ap()

    with tile.TileContext(nc) as tc, ExitStack() as ctx:
        # ---- persistent SBUF pools --------------------------------------
        qk_pool = ctx.enter_context(tc.tile_pool(name="qk", bufs=1))
        v_pool = ctx.enter_context(tc.tile_pool(name="v", bufs=1))
        ot_pool = ctx.enter_context(tc.tile_pool(name="otf", bufs=1))
        const_pool = ctx.enter_context(tc.tile_pool(name="const", bufs=1))
        w_pool = ctx.enter_context(tc.tile_pool(name="wqkv", bufs=1))
        wo_pool = ctx.enter_context(tc.tile_pool(name="wo", bufs=1))
        x_pool = ctx.enter_context(tc.tile_pool(name="xs", bufs=8))
        cs_pool = ctx.enter_context(tc.tile_pool(name="cs", bufs=1))
        t_pool = ctx.enter_context(tc.tile_pool(name="rott", bufs=2))
        p_pool = ctx.enter_context(tc.tile_pool(name="pexp", bufs=4))
        e_pool = ctx.enter_context(tc.tile_pool(name="epi", bufs=1))
        st_pool = ctx.enter_context(tc.tile_pool(name="stg3", bufs=2))
        # single PSUM pool; tags share the 8 banks across phases:
        #   A: proj accum [128,512] + scores pair [128,1024]   (2x2 banks)
        #   B: rotary pswap out + attn AV accum                (2 banks)
        #   C: rotary bdiag out + out-proj accum               (2 banks)
        ps = ctx.enter_context(tc.tile_pool(name="ps", bufs=1, space="PSUM"))

        # q/k as single [128, hp, T] tiles so rotary ops can span all 4
        # head-pairs in one instruction.
        qtb = qk_pool.tile([128, 4, T], F16, name="qtb")
        ktb = qk_pool.tile([128, 4, T], F16, name="ktb")
        v_s = [v_pool.tile([128, NH * 65], F16, name=f"vt{j}")
               for j in range(NTS)]
        otf = [ot_pool.tile([128, T], F16, name=f"otf{j}") for j in range(4)]

        # weights first: they gate the first projection matmuls. Spread the
        # startup DMAs over 4 queues so the first k-groups land early.
        wq_s = [w_pool.tile([128, DH], F16, name=f"wq{k}") for k in range(NKC)]
        wk_s = [w_pool.tile([128, DH], F16, name=f"wk{k}") for k in range(NKC)]
        wv_s = [w_pool.tile([128, DH], F16, name=f"wv{k}") for k in range(NKC)]
        qs = [nc.sync, nc.scalar]
        for k in range(NKC):
            ksl = slice(k * 128, (k + 1) * 128)
            qs[k % 2].dma_start(wq_s[k][:], wqt[ksl, :])
        for k in range(NKC):
            ksl = slice(k * 128, (k + 1) * 128)
            qs[k % 2].dma_start(wk_s[k][:], wkt[ksl, :])
            qs[(k + 1) % 2].dma_start(wv_s[k][:], wvt[ksl, :])
        pswap = const_pool.tile([128, 128], F16, name="pswap_s")
        bdiag = const_pool.tile([128, 128], F16, name="bdiag_s")
        trif = const_pool.tile([128, 256], F16, name="trif_s")
        nc.scalar.dma_start(pswap[:], pswapd[:])
        nc.scalar.dma_start(bdiag[:], bdiagd[:])
        nc.sync.dma_start(trif[:], trid[:])
        wob = [wo_pool.tile([128, D], F16, name=f"wob{k}") for k in range(4)]
        for k4 in range(4):
            nc.scalar.dma_start(wob[k4][:], wot[k4 * 128:(k4 + 1) * 128, :])
        epsb = const_pool.tile([128, 1], F32, name="epsb")
        nc.vector.memset(epsb[:], 8.0 * EPS)
        # Every ACT function this kernel uses ({Ln, Exp, Copy}) lives in the
        # single `natural_log_exp_and_others` table: the rsqrt of the RMS
        # norm is exp(-0.5*ln(v)), so no ACT table ever reloads and rotary
        # can overlap the previous block's softmax freely.
        onescb = const_pool.tile([128, NH], F16, name="onescb")
        nc.vector.memset(onescb[:], 1.0)

        # ---------------- emission helpers ------------------------------
        def emit_x_dma(tt):
            tsl = slice(tt * 512, (tt + 1) * 512)
            xts = []
            for k in range(NKC):
                xc = x_pool.tile([128, 512], F16, name="xc", tag="xc")
                nc.gpsimd.dma_start(xc[:], xt[k * 128:(k + 1) * 128, tsl])
                xts.append(xc)
            return xts

        def py_chain(pib, dt_, tag="C"):
            """Out-projection of one dout chunk of i-block pib (PE filler)."""
            dsl = slice(dt_ * 128, (dt_ + 1) * 128)
            psl = slice(pib * 512, (pib + 1) * 512)
            py = ps.tile([128, 512], F32, name="py", tag=tag, bufs=2)
            for k4 in range(4):
                nc.tensor.matmul(py[:], wob[k4][:, dsl], otf[k4][:, psl],
                                 start=(k4 == 0), stop=(k4 == 3))
            st = st_pool.tile([128, 512], F16, name="st", tag="st", bufs=3)
            nc.vector.tensor_copy(st[:], py[:])
            nc.sync.dma_start(yt[dsl, psl], st[:])

        def proj_closures(tt, xts):
            """PE projection k-groups for block tt, split (q, k, v); q/k
            copies trail on DVE, v copies on ACT."""
            tsl = slice(tt * 512, (tt + 1) * 512)
            qk_items = {"q": [], "k": []}
            for (nm, wsrc, dstb) in (("q", wq_s, qtb), ("k", wk_s, ktb)):
                for hp in range(4):
                    def g(wsrc=wsrc, dstb=dstb, hp=hp):
                        jsl = slice(hp * 128, (hp + 1) * 128)
                        pp = ps.tile([128, 1024], F32, name="pp", tag="A",
                                     bufs=2)
                        for k in range(NKC):
                            nc.tensor.matmul(
                                pp[:, 0:512], wsrc[k][:, jsl], xts[k][:],
                                start=(k == 0), stop=(k == NKC - 1))
                        nc.vector.tensor_copy(dstb[:, hp, tsl], pp[:, 0:512])
                    qk_items[nm].append(g)
            v_items = []
            for ts_ in range(4):
                def g(ts_=ts_):
                    ci = tt * 4 + ts_
                    pv = ps.tile([128, 1024], F32, name="pv", tag="A", bufs=2)
                    for k in range(NKC):
                        nc.tensor.matmul(
                            pv[:, 0:512],
                            xts[k][:, ts_ * 128:(ts_ + 1) * 128], wv_s[k][:],
                            start=(k == 0), stop=(k == NKC - 1))
                    v3 = v_s[ci].rearrange("p (h e) -> p h e", h=NH)
                    nc.scalar.copy(
                        v3[:, :, 0:64],
                        pv[:, 0:512].rearrange("p (h e) -> p h e", h=NH))
                    nc.vector.tensor_copy(v3[:, :, 64:65],
                                          onescb[:].unsqueeze(-1))
                v_items.append(g)
            return qk_items["q"], qk_items["k"], v_items

        def rot_closures(tt):
            """Rotary + RMS-norm for q and k of block tt, as wide 4-hp ops.
            Per nm: sq -> {pswap MM, xs*s2 -> fp16}, {bdiag MM, ln} per hp
            -> exp(-ln/2) -> x*c2 -> +xs -> *s1. Returns (q_items, k_items).
            rsqrt via ln+exp keeps the ACT on one function table."""
            tsl = slice(tt * 512, (tt + 1) * 512)
            c2w = cs_pool.tile([128, 4, 512], F16, name="c2w", tag="c2w",
                               bufs=2)
            s2w = cs_pool.tile([128, 4, 512], F32, name="s2w", tag="s2w",
                               bufs=2)
            nc.sync.dma_start(c2w[:], c2d[:, :, tsl])
            nc.scalar.dma_start(s2w[:], s2d[:, :, tsl])
            items = {}

            for nm, big in (("q", qtb), ("k", ktb)):
                st_ = {}
                out = []

                def g_sq(big=big, st_=st_):
                    sqb = t_pool.tile([128, 4, 512], F16, name="sqb",
                                      tag="sqb", bufs=2)
                    nc.vector.tensor_mul(sqb[:], big[:, :, tsl],
                                         big[:, :, tsl])
                    st_["sqb"] = sqb
                    st_["xsb"] = t_pool.tile([128, 4, 512], F16, name="xsb",
                                             tag="xsb", bufs=2)
                    st_["lnv"] = e_pool.tile([128, 4, 512], F32, name="lnv",
                                             tag="lnv", bufs=2)
                out.append(g_sq)

                for hp in range(4):
                    def g_ps(big=big, st_=st_, hp=hp):
                        xs_ = ps.tile([128, 512], F32, name="xs", tag="B",
                                      bufs=2)
                        nc.tensor.matmul(xs_[:], pswap[:], big[:, hp, tsl],
                                         start=True, stop=True)
                        # PSUM -> fp16 SBUF bridge: (pswap x) * sin
                        nc.vector.tensor_mul(st_["xsb"][:, hp, :], xs_[:],
                                             s2w[:, hp, :])
                    out.append(g_ps)

                    def g_ms(big=big, st_=st_, hp=hp):
                        ms = ps.tile([128, 512], F32, name="ms", tag="C",
                                     bufs=2)
                        nc.tensor.matmul(ms[:], bdiag[:],
                                         st_["sqb"][:, hp, :],
                                         start=True, stop=True)
                        nc.scalar.activation(
                            st_["lnv"][:, hp, :], ms[:], AF.Ln,
                            scale=0.125, bias=epsb[:])
                    out.append(g_ms)

                def g_nexp(st_=st_):
                    # s1 = exp(-0.5 * ln v) = rsqrt(v), one wide ACT op
                    s1h = t_pool.tile([128, 4, 512], F16, name="s1h",
                                      tag="s1h", bufs=2)
                    nc.scalar.activation(s1h[:], st_["lnv"][:], AF.Exp,
                                         scale=-0.5)
                    st_["s1h"] = s1h
                out.append(g_nexp)

                def g_cmul(big=big):
                    nc.vector.tensor_mul(big[:, :, tsl], big[:, :, tsl],
                                         c2w[:])
                out.append(g_cmul)

                def g_add(big=big, st_=st_):
                    nc.vector.tensor_add(big[:, :, tsl], big[:, :, tsl],
                                         st_["xsb"][:])
                out.append(g_add)

                def g_s1mul(big=big, st_=st_):
                    nc.vector.tensor_mul(big[:, :, tsl], big[:, :, tsl],
                                         st_["s1h"][:])
                out.append(g_s1mul)
                items[nm] = out
            return items["q"], items["k"]

        def attn_closures(tt):
            """Software-pipelined attention for i-block tt: scores(jt+1) is
            emitted before AV(jt) so interleaved PE work covers exp."""
            ib = tt
            isl = slice(tt * 512, (tt + 1) * 512)
            nj = 4 * ib + 4

            def emit_sc(hp, jt):
                jsl = slice(jt * 128, (jt + 1) * 128)
                c_ = jt - 4 * ib          # >=0 on diagonal chunks
                off = 128 * c_ if c_ >= 0 else 0
                osc = off if off <= 384 else 384
                sc = ps.tile([128, 1024], F32, name="sc", tag="A", bufs=2)
                for h2 in range(2):
                    ho = h2 * 64
                    nc.tensor.matmul(
                        sc[:, 512 * h2 + osc:512 * h2 + 512],
                        ktb[ho:ho + 64, hp, jsl],
                        qtb[ho:ho + 64, hp, ib * 512 + osc:ib * 512 + 512],
                        start=True, stop=True)
                return sc, off, c_

            def warm_mm(box, n):
                # keep-warm padding: fp16 matmuls into unused psum rows
                # 96-127 of the AV accumulator; they hold the HAM clock
                # gate at K=8/8 while ACT paces the softmax.
                for _ in range(n):
                    # start/stop False: ride the open AV accumulation group
                    nc.tensor.matmul(box["ot"][0][96:128, 0:512],
                                     v_s[0][:, 0:32], v_s[0][:, 0:512],
                                     start=False, stop=False,
                                     tile_position=(0, 96))

            def emit_av(hp, box, jt, p, off):
                for h2 in range(2):
                    h = 2 * hp + h2
                    nc.tensor.matmul(
                        box["ot"][h2][0:65, off:512],
                        v_s[jt][:, 65 * h:65 * h + 65],
                        p[:, 512 * h2 + off:512 * h2 + 512],
                        start=(jt == 0), stop=(jt == nj - 1))

            out = []
            for hp in range(4):
                box = {}

                def c_start(hp=hp, box=box):
                    box["ot"] = [ps.tile([128, 512], F32, name="otp",
                                         tag="B", bufs=2) for _ in range(2)]
                    box["nxt"] = emit_sc(hp, 0)
                    box["avq"] = []
                out.append(c_start)

                for jt in range(nj):
                    def c_item(hp=hp, jt=jt, box=box):
                        sc, off, c_ = box["nxt"]
                        sc3 = sc.rearrange("p (h e) -> p h e", h=2)
                        p = p_pool.tile([128, 1024], F16, name="p", tag="p")
                        p3 = p.rearrange("p (h e) -> p h e", h=2)
                        nc.scalar.activation(p3[:, :, off:512],
                                             sc3[:, :, off:512], AF.Exp,
                                             scale=1.0)
                        if c_ >= 0:
                            # multiplicative causal mask (0/1) post-exp in
                            # fp16 (2x DVE rate vs the old f32 PSUM add).
                            # The AV ones-row consumes masked p, so the
                            # denominator is right.
                            nc.vector.tensor_mul(
                                p3[:, :, off:off + 128],
                                p3[:, :, off:off + 128],
                                trif[:].rearrange("p (h e) -> p h e", h=2))
                        if jt + 1 < nj:
                            box["nxt"] = emit_sc(hp, jt + 1)
                        # AV runs one pipeline step behind its exp so the
                        # PE never waits on the ACT queue.
                        box["avq"].append((jt, p, off))
                        if len(box["avq"]) > 1:
                            emit_av(hp, box, *box["avq"].pop(0))
                    out.append(c_item)

                def c_epi(hp=hp, box=box):
                    while box["avq"]:
                        emit_av(hp, box, *box["avq"].pop(0))
                    # epilogue: divide by the denominator row (psum row 64).
                    for h2 in range(2):
                        op = box["ot"][h2]
                        rden = e_pool.tile([1, 512], F32, name="rden",
                                           tag="rden")
                        nc.vector.tensor_copy(rden[:], op[64:65, :])
                        nc.vector.reciprocal_approx_fast(out=rden[:],
                                                         in_=rden[:])
                        rb = e_pool.tile([64, 512], F32, name="rb", tag="rb")
                        nc.gpsimd.partition_broadcast(rb[:], rden[:],
                                                      channels=64)
                        ho = h2 * 64
                        nc.vector.tensor_mul(otf[hp][ho:ho + 64, isl],
                                             op[0:64, :], rb[:])
                out.append(c_epi)
            return out

        def interleave(a, b):
            ia = ib_ = 0
            while ib_ < min(3, len(b)):     # front-load PE cover
                b[ib_]()
                ib_ += 1
            while ia < len(a) or ib_ < len(b):
                if ib_ >= len(b) or (ia < len(a)
                                     and ia * (len(b) - 3) <= (ib_ - 3)
                                     * max(1, len(a))):
                    a[ia]()
                    ia += 1
                else:
                    b[ib_]()
                    ib_ += 1

        # ---------------- schedule --------------------------------------
        # rotary of block tt+1 rides in b_items during attention of block
        # tt (right after the q/k projection copies it needs), so the PE
        # never drains while the DVE runs the rotary chain.
        xts = emit_x_dma(0)
        pq, pk, pv = proj_closures(0, xts)
        for f in pq + pk + pv:
            f()
        rq0, rk0 = rot_closures(0)
        pending_py = []
        for tt in range(NTT):
            a_items = attn_closures(tt)
            if tt == 0:
                a_items = rq0 + rk0 + a_items
            b_items = []
            if tt + 1 < NTT:
                xts = emit_x_dma(tt + 1)
                pq, pk, pv = proj_closures(tt + 1, xts)
                rq, rk = rot_closures(tt + 1)
                b_items += pq + rq + pk + rk + pv
            if tt == NTT - 1:
                take = [p_ for p_ in pending_py if p_[0] <= tt - 1]
            else:
                take = [p_ for p_ in pending_py if p_[0] == tt - 2]
            for p_ in take:
                pending_py.remove(p_)
                b_items.append(lambda p_=p_: py_chain(*p_))
            interleave(a_items, b_items)
            pending_py += [(tt, d) for d in range(8)]
        wps = ps.tile([128, 512], F32, name="wps", tag="B", bufs=2)
        nc.tensor.matmul(wps[96:128, 0:512], v_s[0][:, 0:32],
                         v_s[0][:, 0:512], start=True, stop=False,
                         tile_position=(0, 96))
        for n_, (pib, d) in enumerate(pending_py):
            py_chain(pib, d, tag=("C" if n_ % 2 == 0 else "A"))
            for _ in range(3):
                nc.tensor.matmul(wps[96:128, 0:512], v_s[0][:, 0:32],
                                 v_s[0][:, 0:512], start=False, stop=False,
                                 tile_position=(0, 96))
        nc.tensor.matmul(wps[96:128, 0:512], v_s[0][:, 0:32],
                         v_s[0][:, 0:512], start=False, stop=True,
                         tile_position=(0, 96))
    return nc


# ---------------- host-side tables & shard prep -------------------------

def host_tables(T: int = 2048):
    n = HD // 4
    af = (1.0 / 1024) ** np.linspace(0, 1, n, dtype=np.float32)
    af = np.concatenate([af, np.zeros(n, np.float32)])  # [32]
    theta = np.outer(np.arange(T, dtype=np.float32), af)  # [T, 32]
    cosT = np.cos(theta).T.astype(np.float32)  # [32, T]
    sinT = np.sin(theta).T.astype(np.float32)
    c2 = np.tile(cosT, (4, 1))                             # [128, T]
    s2 = np.tile(np.concatenate([sinT, -sinT], 0), (2, 1))  # [128, T]
    c2r = np.ascontiguousarray(
        np.broadcast_to(c2[:, None, :], (128, 4, T))).astype(np.float16)
    s2r = np.ascontiguousarray(
        np.broadcast_to(s2[:, None, :], (128, 4, T))).astype(np.float32)
    km = np.arange(128)
    pswap = (km[:, None] == (km[None, :] ^ 32)).astype(np.float16)
    bdiag = ((km[:, None] // 64) == (km[None, :] // 64)).astype(np.float16)
    r_ = np.arange(128)[:, None]
    c_ = np.arange(128)[None, :]
    tri1 = np.where(c_ >= r_, 1.0, 0.0).astype(np.float16)
    tri = np.concatenate([tri1, tri1], axis=1)             # [128, 256]
    return {"c2": c2r, "s2": s2r,
            "pswap": pswap, "bdiag": bdiag,
            "trimask": np.ascontiguousarray(tri)}


def core_inputs(x, wq, wk, wv, wo, core: int, T: int = 2048):
    b, g = core % 4, core // 4
    sl = slice(g * DH, (g + 1) * DH)
    m = {
        "xt": np.ascontiguousarray(np.asarray(x[b]).T).astype(np.float16),
        "wqt": np.ascontiguousarray(
            np.asarray(wq)[sl, :].T).astype(np.float16),
        "wkt": np.ascontiguousarray(
            np.asarray(wk)[sl, :].T).astype(np.float16),
        "wvt": np.ascontiguousarray(
            np.asarray(wv)[sl, :].T).astype(np.float16),
        "wot": np.ascontiguousarray(
            np.asarray(wo)[:, sl].T).astype(np.float16),
    }
    m.update(host_tables(T))
    return m


_CACHE = {}


def _get_nc(T: int = 2048):
    key = ("nc", T)
    if key not in _CACHE:
        nc = bacc.Bacc("TRN2", target_bir_lowering=False, debug=False)
        build_kernel(nc, T)
        nc.compile()
        _CACHE[key] = nc
    return _CACHE[key]


def kernel(x, wq, wk, wv, wo, mask=None):
    from concourse import bass_utils
    nc = _get_nc(2048)
    in_maps = [core_inputs(x, wq, wk, wv, wo, c) for c in range(8)]
    res = bass_utils.run_bass_kernel_spmd(nc, in_maps, list(range(8)))
    outs = [np.asarray(res.results[c]["yt"]).astype(np.float32)
            for c in range(8)]
    out = np.empty((4, 2048, 1024), np.float32)
    for b in range(4):
        out[b] = (outs[b] + outs[b + 4]).T
    return out


# revision 29
# speedup vs baseline: 1.1618x; 1.1618x over previous
"""Trainium2 Bass kernel: MultiHeadAttention with QK-RMSNorm + partial rotary,
causal softmax. B=4, T=2048, D=1024, H=16, HD=64, fp32 reference.

Sharding: 8 cores = 4 batches x 2 head-groups (8 heads each). Each core:
  - QKV projections for its batch, restricted to its 512 head-dims
  - causal attention for its 8 heads
  - partial output projection (its 512 contraction dims, all 1024 outputs)
Host sums the two head-group partials per batch (the all-reduce) and
transposes back.

v4 design (from v3): all-fp16 matmul data + wide-op rotary.
  - fp16 everywhere on the MM path (x, w, q, k, v, wo, rotary tables):
    same PE rate as bf16/f32r warm (1 cyc/row), no f32r cold 4x penalty,
    FWL on weight loads, half DMA bytes, 8-bit-mantissa accuracy.
  - q/k live in single [128, 4, T] tiles so the rotary/RMS-norm chain runs
    as wide [128,4,512] DVE ops (one per step instead of per-head-pair),
    cutting DVE busy in the rotary window ~2x; 16-bit DVE ops run 2x.
  - PSUM->SBUF bridge: xs*s2 writes an fp16 SBUF tile so the following
    add/mul stay 16-bit SBUF-only.
  - epilogue: reciprocal reads the denominator row straight from PSUM.
  - diagonal score strips computed at their true width (min N=128).
  - q/k projection copies on DVE, v copies on ACT (engine balance).
"""

import numpy as np
from contextlib import ExitStack

import concourse.bass as bass
import concourse.tile as tile
import concourse.mybir as mybir
from concourse import bacc

F32 = mybir.dt.float32
F16 = mybir.dt.float16
AF = mybir.ActivationFunctionType
MUL = mybir.AluOpType.mult
ADD = mybir.AluOpType.add

D = 1024   # model dim
DH = 512   # head-group width per core (8 heads x 64)
NH = 8     # heads per core
HD = 64    # head dim
NKC = D // 128   # k-chunks over model dim
EPS = 1e-6


def build_kernel(nc: bass.Bass, T: int = 2048):
    NTT = T // 512     # 512-wide t/i blocks
    NTS = T // 128     # 128-wide t/j chunks

    xt = nc.dram_tensor("xt", [D, T], F16, kind="ExternalInput").ap()
    wqt = nc.dram_tensor("wqt", [D, DH], F16, kind="ExternalInput").ap()
    wkt = nc.dram_tensor("wkt", [D, DH], F16, kind="ExternalInput").ap()
    wvt = nc.dram_tensor("wvt", [D, DH], F16, kind="ExternalInput").ap()
    wot = nc.dram_tensor("wot", [DH, D], F16, kind="ExternalInput").ap()
    c2d = nc.dram_tensor("c2", [128, 4, T], F16, kind="ExternalInput").ap()
    s2d = nc.dram_tensor("s2", [128, 4, T], F32, kind="ExternalInput").ap()
    pswapd = nc.dram_tensor("pswap", [128, 128], F16, kind="ExternalInput").ap()
    bdiagd = nc.dram_tensor("bdiag", [128, 128], F16, kind="ExternalInput").ap()
    trid = nc.dram_tensor("trimask", [128, 256], F16, kind="ExternalInput").ap()
    yt = nc.dram_tensor("yt", [D, T], F16, kind="ExternalOutput").ap()

    with tile.TileContext(nc) as tc, ExitStack() as ctx:
        # ---- persistent SBUF pools --------------------------------------
        qk_pool = ctx.enter_context(tc.tile_pool(name="qk", bufs=1))
        v_pool = ctx.enter_context(tc.tile_pool(name="v", bufs=1))
        ot_pool = ctx.enter_context(tc.tile_pool(name="otf", bufs=1))
        const_pool = ctx.enter_context(tc.tile_pool(name="const", bufs=1))
        w_pool = ctx.enter_context(tc.tile_pool(name="wqkv", bufs=1))
        wo_pool = ctx.enter_context(tc.tile_pool(name="wo", bufs=1))
        x_pool = ctx.enter_context(tc.tile_pool(name="xs", bufs=8))
        cs_pool = ctx.enter_context(tc.tile_pool(name="cs", bufs=1))
        t_pool = ctx.enter_context(tc.tile_pool(name="rott", bufs=2))
        p_pool = ctx.enter_context(tc.tile_pool(name="pexp", bufs=4))
        e_pool = ctx.enter_context(tc.tile_pool(name="epi", bufs=1))
        st_pool = ctx.enter_context(tc.tile_pool(name="stg3", bufs=2))
        # single PSUM pool; tags share the 8 banks across phases:
        #   A: proj accum [128,512] + scores pair [128,1024]   (2x2 banks)
        #   B: rotary pswap out + attn AV accum                (2 banks)
        #   C: rotary bdiag out + out-proj accum               (2 banks)
        ps = ctx.enter_context(tc.tile_pool(name="ps", bufs=1, space="PSUM"))

        # q/k as single [128, hp, T] tiles so rotary ops can span all 4
        # head-pairs in one instruction.
        qtb = qk_pool.tile([128, 4, T], F16, name="qtb")
        ktb = qk_pool.tile([128, 4, T], F16, name="ktb")
        v_s = [v_pool.tile([128, NH * 65], F16, name=f"vt{j}")
               for j in range(NTS)]
        otf = [ot_pool.tile([128, T], F16, name=f"otf{j}") for j in range(4)]

        # weights first: they gate the first projection matmuls. Spread the
        # startup DMAs over 4 queues so the first k-groups land early.
        wq_s = [w_pool.tile([128, DH], F16, name=f"wq{k}") for k in range(NKC)]
        wk_s = [w_pool.tile([128, DH], F16, name=f"wk{k}") for k in range(NKC)]
        wv_s = [w_pool.tile([128, DH], F16, name=f"wv{k}") for k in range(NKC)]
        qs = [nc.sync, nc.scalar]
        for k in range(NKC):
            ksl = slice(k * 128, (k + 1) * 128)
            qs[k % 2].dma_start(wq_s[k][:], wqt[ksl, :])
        for k in range(NKC):
            ksl = slice(k * 128, (k + 1) * 128)
            qs[k % 2].dma_start(wk_s[k][:], wkt[ksl, :])
            qs[(k + 1) % 2].dma_start(wv_s[k][:], wvt[ksl, :])
        pswap = const_pool.tile([128, 128], F16, name="pswap_s")
        bdiag = const_pool.tile([128, 128], F16, name="bdiag_s")
        trif = const_pool.tile([128, 256], F16, name="trif_s")
        nc.scalar.dma_start(pswap[:], pswapd[:])
        nc.scalar.dma_start(bdiag[:], bdiagd[:])
        nc.sync.dma_start(trif[:], trid[:])
        wob = [wo_pool.tile([128, D], F16, name=f"wob{k}") for k in range(4)]
        for k4 in range(4):
            nc.scalar.dma_start(wob[k4][:], wot[k4 * 128:(k4 + 1) * 128, :])
        epsb = const_pool.tile([128, 1], F32, name="epsb")
        nc.vector.memset(epsb[:], 8.0 * EPS)
        onesf = const_pool.tile([128, 1], F32, name="onesf")
        nc.vector.memset(onesf[:], 1.0)
        # ACT-ordering tokens: the rotary Ln/Exp bunch of block tt+1 is
        # anchored mid-way through block tt's softmax stream (after hp1's
        # last exp, before hp2's first), so the ACT table swaps exactly
        # twice per block even though rotary overlaps attention.
        tstate = {}
        onescb = const_pool.tile([128, NH], F16, name="onescb")
        nc.vector.memset(onescb[:], 1.0)

        # ---------------- emission helpers ------------------------------
        def emit_x_dma(tt):
            tsl = slice(tt * 512, (tt + 1) * 512)
            xts = []
            for k in range(NKC):
                xc = x_pool.tile([128, 512], F16, name="xc", tag="xc")
                nc.gpsimd.dma_start(xc[:], xt[k * 128:(k + 1) * 128, tsl])
                xts.append(xc)
            return xts

        def py_chain(pib, dt_, tag="C"):
            """Out-projection of one dout chunk of i-block pib (PE filler)."""
            dsl = slice(dt_ * 128, (dt_ + 1) * 128)
            psl = slice(pib * 512, (pib + 1) * 512)
            py = ps.tile([128, 512], F32, name="py", tag=tag, bufs=2)
            for k4 in range(4):
                nc.tensor.matmul(py[:], wob[k4][:, dsl], otf[k4][:, psl],
                                 start=(k4 == 0), stop=(k4 == 3))
            st = st_pool.tile([128, 512], F16, name="st", tag="st", bufs=3)
            nc.vector.tensor_copy(st[:], py[:])
            nc.sync.dma_start(yt[dsl, psl], st[:])

        def proj_closures(tt, xts):
            """PE projection k-groups for block tt, split (q, k, v); q/k
            copies trail on DVE, v copies on ACT."""
            tsl = slice(tt * 512, (tt + 1) * 512)
            qk_items = {"q": [], "k": []}
            for (nm, wsrc, dstb) in (("q", wq_s, qtb), ("k", wk_s, ktb)):
                for hp in range(4):
                    def g(wsrc=wsrc, dstb=dstb, hp=hp):
                        jsl = slice(hp * 128, (hp + 1) * 128)
                        pp = ps.tile([128, 1024], F32, name="pp", tag="A",
                                     bufs=2)
                        for k in range(NKC):
                            nc.tensor.matmul(
                                pp[:, 0:512], wsrc[k][:, jsl], xts[k][:],
                                start=(k == 0), stop=(k == NKC - 1))
                        nc.vector.tensor_copy(dstb[:, hp, tsl], pp[:, 0:512])
                    qk_items[nm].append(g)
            v_items = []
            for ts_ in range(4):
                def g(ts_=ts_):
                    ci = tt * 4 + ts_
                    pv = ps.tile([128, 1024], F32, name="pv", tag="A", bufs=2)
                    for k in range(NKC):
                        nc.tensor.matmul(
                            pv[:, 0:512],
                            xts[k][:, ts_ * 128:(ts_ + 1) * 128], wv_s[k][:],
                            start=(k == 0), stop=(k == NKC - 1))
                    v3 = v_s[ci].rearrange("p (h e) -> p h e", h=NH)
                    nc.scalar.copy(
                        v3[:, :, 0:64],
                        pv[:, 0:512].rearrange("p (h e) -> p h e", h=NH))
                    nc.vector.tensor_copy(v3[:, :, 64:65],
                                          onescb[:].unsqueeze(-1))
                v_items.append(g)
            return qk_items["q"], qk_items["k"], v_items

        def rot_closures(tt):
            """Rotary + RMS-norm for q and k of block tt, as wide 4-hp ops.
            Split into {pre, bunch, post}: pre = squares + pswap MMs + sin
            bridges (DVE/PE only); bunch = ALL bdiag MMs + Ln + exp(-ln/2)
            (the only rotary ACT ops, emitted as one token-gated group so
            the ACT table swaps just twice per block); post = cos-mul, add,
            norm-mul (DVE only)."""
            tsl = slice(tt * 512, (tt + 1) * 512)
            c2w = cs_pool.tile([128, 4, 512], F16, name="c2w", tag="c2w",
                               bufs=2)
            s2w = cs_pool.tile([128, 4, 512], F32, name="s2w", tag="s2w",
                               bufs=2)
            nc.sync.dma_start(c2w[:], c2d[:, :, tsl])
            nc.scalar.dma_start(s2w[:], s2d[:, :, tsl])
            sts = {"q": {}, "k": {}}
            pre = {"q": [], "k": []}
            post = {"q": [], "k": []}

            for nm, big in (("q", qtb), ("k", ktb)):
                st_ = sts[nm]
                out = pre[nm]

                def g_sq(big=big, st_=st_):
                    sqb = t_pool.tile([128, 4, 512], F16, name="sqb",
                                      tag="sqb", bufs=2)
                    nc.vector.tensor_mul(sqb[:], big[:, :, tsl],
                                         big[:, :, tsl])
                    st_["sqb"] = sqb
                    st_["xsb"] = t_pool.tile([128, 4, 512], F16, name="xsb",
                                             tag="xsb", bufs=2)
                    st_["lnv"] = e_pool.tile([128, 4, 512], F32, name="lnv",
                                             tag="lnv", bufs=2)
                out.append(g_sq)

                for hp in range(4):
                    def g_ps(big=big, st_=st_, hp=hp):
                        xs_ = ps.tile([128, 512], F32, name="xs", tag="B",
                                      bufs=2)
                        nc.tensor.matmul(xs_[:], pswap[:], big[:, hp, tsl],
                                         start=True, stop=True)
                        # PSUM -> fp16 SBUF bridge: (pswap x) * sin
                        nc.vector.tensor_mul(st_["xsb"][:, hp, :], xs_[:],
                                             s2w[:, hp, :])
                    out.append(g_ps)

                out = post[nm]

                def g_cmul(big=big):
                    nc.vector.tensor_mul(big[:, :, tsl], big[:, :, tsl],
                                         c2w[:])
                out.append(g_cmul)

                def g_add(big=big, st_=st_):
                    nc.vector.tensor_add(big[:, :, tsl], big[:, :, tsl],
                                         st_["xsb"][:])
                out.append(g_add)

                def g_s1mul(big=big, st_=st_):
                    nc.vector.tensor_mul(big[:, :, tsl], big[:, :, tsl],
                                         st_["s1h"][:])
                out.append(g_s1mul)

            def bunch():
                # epsw carries the token dep on the last pre-bunch exp, so
                # every Ln/Exp here lands contiguously on the ACT queue.
                if tstate.get("tok1") is not None:
                    epsw = e_pool.tile([128, 1], F32, name="epsw",
                                       tag="epsw", bufs=2)
                    nc.vector.scalar_tensor_tensor(
                        epsw[:], tstate["tok1"][:], 0.0, epsb[:], MUL, ADD)
                else:
                    epsw = epsb
                for nm in ("q", "k"):
                    st_ = sts[nm]
                    for hp in range(4):
                        ms = ps.tile([128, 512], F32, name="ms", tag="C",
                                     bufs=2)
                        nc.tensor.matmul(ms[:], bdiag[:],
                                         st_["sqb"][:, hp, :],
                                         start=True, stop=True)
                        nc.scalar.activation(
                            st_["lnv"][:, hp, :], ms[:], AF.Ln,
                            scale=0.125, bias=epsw[:])
                    # s1 = exp(-0.5 ln v) = rsqrt(v), one wide ACT op
                    s1h = t_pool.tile([128, 4, 512], F16, name="s1h",
                                      tag="s1h", bufs=2)
                    acc = None
                    if nm == "k":
                        acc = e_pool.tile([128, 1], F32, name="tok2",
                                          tag="tok2", bufs=2)
                    nc.scalar.activation(s1h[:], st_["lnv"][:], AF.Exp,
                                         scale=-0.5,
                                         accum_out=(acc[:] if acc is not
                                                    None else None))
                    st_["s1h"] = s1h
                # post-bunch exps wait on the last rotary ACT op
                sone = e_pool.tile([128, 1], F32, name="sone", tag="sone",
                                   bufs=2)
                nc.vector.scalar_tensor_tensor(
                    sone[:], acc[:], 0.0, onesf[:], MUL, ADD)
                tstate["sone"] = sone

            return pre["q"], pre["k"], bunch, post["q"] + post["k"]

        def attn_closures(tt):
            """Software-pipelined attention for i-block tt: scores(jt+1) is
            emitted before AV(jt) so interleaved PE work covers exp."""
            ib = tt
            isl = slice(tt * 512, (tt + 1) * 512)
            nj = 4 * ib + 4

            def emit_sc(hp, jt):
                jsl = slice(jt * 128, (jt + 1) * 128)
                c_ = jt - 4 * ib          # >=0 on diagonal chunks
                off = 128 * c_ if c_ >= 0 else 0
                osc = off if off <= 384 else 384
                sc = ps.tile([128, 1024], F32, name="sc", tag="A", bufs=2)
                for h2 in range(2):
                    ho = h2 * 64
                    nc.tensor.matmul(
                        sc[:, 512 * h2 + osc:512 * h2 + 512],
                        ktb[ho:ho + 64, hp, jsl],
                        qtb[ho:ho + 64, hp, ib * 512 + osc:ib * 512 + 512],
                        start=True, stop=True)
                return sc, off, c_

            def warm_mm(box, n):
                # keep-warm padding: fp16 matmuls into unused psum rows
                # 96-127 of the AV accumulator; they hold the HAM clock
                # gate at K=8/8 while ACT paces the softmax.
                for _ in range(n):
                    # start/stop False: ride the open AV accumulation group
                    nc.tensor.matmul(box["ot"][0][96:128, 0:512],
                                     v_s[0][:, 0:32], v_s[0][:, 0:512],
                                     start=False, stop=False,
                                     tile_position=(0, 96))

            def emit_av(hp, box, jt, p, off):
                for h2 in range(2):
                    h = 2 * hp + h2
                    nc.tensor.matmul(
                        box["ot"][h2][0:65, off:512],
                        v_s[jt][:, 65 * h:65 * h + 65],
                        p[:, 512 * h2 + off:512 * h2 + 512],
                        start=(jt == 0), stop=(jt == nj - 1))

            out = []
            for hp in range(4):
                box = {}

                def c_start(hp=hp, box=box):
                    box["ot"] = [ps.tile([128, 512], F32, name="otp",
                                         tag="B", bufs=2) for _ in range(2)]
                    box["nxt"] = emit_sc(hp, 0)
                    box["avq"] = []
                out.append(c_start)

                for jt in range(nj):
                    def c_item(hp=hp, jt=jt, box=box):
                        sc, off, c_ = box["nxt"]
                        sc3 = sc.rearrange("p (h e) -> p h e", h=2)
                        p = p_pool.tile([128, 1024], F16, name="p", tag="p")
                        p3 = p.rearrange("p (h e) -> p h e", h=2)
                        acc = None
                        if hp == 1 and jt == nj - 1:
                            # token: the next rotary ACT bunch anchors here
                            acc = e_pool.tile([128, 1], F32, name="tok1",
                                              tag="tok1", bufs=2)
                            tstate["tok1"] = acc
                        nc.scalar.activation(p3[:, :, off:512],
                                             sc3[:, :, off:512], AF.Exp,
                                             scale=tstate.get("sone",
                                                              onesf)[:],
                                             accum_out=(acc[:] if acc is not
                                                        None else None))
                        if c_ >= 0:
                            # multiplicative causal mask (0/1) post-exp in
                            # fp16 (2x DVE rate vs the old f32 PSUM add).
                            # The AV ones-row consumes masked p, so the
                            # denominator is right.
                            nc.vector.tensor_mul(
                                p3[:, :, off:off + 128],
                                p3[:, :, off:off + 128],
                                trif[:].rearrange("p (h e) -> p h e", h=2))
                        if jt + 1 < nj:
                            box["nxt"] = emit_sc(hp, jt + 1)
                        # AV runs one pipeline step behind its exp so the
                        # PE never waits on the ACT queue.
                        box["avq"].append((jt, p, off))
                        if len(box["avq"]) > 1:
                            emit_av(hp, box, *box["avq"].pop(0))
                    out.append(c_item)

                def c_epi(hp=hp, box=box):
                    while box["avq"]:
                        emit_av(hp, box, *box["avq"].pop(0))
                    # epilogue: divide by the denominator row (psum row 64).
                    for h2 in range(2):
                        op = box["ot"][h2]
                        rden = e_pool.tile([1, 512], F32, name="rden",
                                           tag="rden")
                        nc.vector.tensor_copy(rden[:], op[64:65, :])
                        nc.vector.reciprocal_approx_fast(out=rden[:],
                                                         in_=rden[:])
                        rb = e_pool.tile([64, 512], F32, name="rb", tag="rb")
                        nc.gpsimd.partition_broadcast(rb[:], rden[:],
                                                      channels=64)
                        ho = h2 * 64
                        nc.vector.tensor_mul(otf[hp][ho:ho + 64, isl],
                                             op[0:64, :], rb[:])
                out.append(c_epi)
            return out

        def interleave(a, b):
            ia = ib_ = 0
            while ib_ < min(3, len(b)):     # front-load PE cover
                b[ib_]()
                ib_ += 1
            while ia < len(a) or ib_ < len(b):
                if ib_ >= len(b) or (ia < len(a)
                                     and ia * (len(b) - 3) <= (ib_ - 3)
                                     * max(1, len(a))):
                    a[ia]()
                    ia += 1
                else:
                    b[ib_]()
                    ib_ += 1

        # ---------------- schedule --------------------------------------
        # rotary of block tt+1 rides in b_items during attention of block
        # tt (right after the q/k projection copies it needs), so the PE
        # never drains while the DVE runs the rotary chain. Its ACT bunch
        # is token-anchored between the hp1 and hp2 softmax streams.
        xts = emit_x_dma(0)
        pq, pk, pv = proj_closures(0, xts)
        for f in pq + pk + pv:
            f()
        rq0, rk0, bunch0, post0 = rot_closures(0)
        pending_py = []
        for tt in range(NTT):
            a_items = attn_closures(tt)
            nj = 4 * tt + 4
            half = 2 * (nj + 2)     # attn items for hp0+hp1
            if tt == 0:
                for f in rq0 + rk0 + [bunch0] + post0:
                    f()
            b_pre, b_post = [], []
            if tt + 1 < NTT:
                xts = emit_x_dma(tt + 1)
                pq, pk, pv = proj_closures(tt + 1, xts)
                rq, rk, bunch, post = rot_closures(tt + 1)
                b_pre = pq + rq + pk + rk
                b_post = pv + post
            else:
                bunch = None
            if tt == NTT - 1:
                take = [p_ for p_ in pending_py if p_[0] <= tt - 1]
            else:
                take = [p_ for p_ in pending_py if p_[0] == tt - 2]
            for p_ in take:
                pending_py.remove(p_)
                b_post.append(lambda p_=p_: py_chain(*p_))
            interleave(a_items[:half], b_pre)
            if bunch is not None:
                bunch()
            interleave(a_items[half:], b_post)
            pending_py += [(tt, d) for d in range(8)]
        wps = ps.tile([128, 512], F32, name="wps", tag="B", bufs=2)
        nc.tensor.matmul(wps[96:128, 0:512], v_s[0][:, 0:32],
                         v_s[0][:, 0:512], start=True, stop=False,
                         tile_position=(0, 96))
        for n_, (pib, d) in enumerate(pending_py):
            py_chain(pib, d, tag=("C" if n_ % 2 == 0 else "A"))
            for _ in range(3):
                nc.tensor.matmul(wps[96:128, 0:512], v_s[0][:, 0:32],
                                 v_s[0][:, 0:512], start=False, stop=False,
                                 tile_position=(0, 96))
        nc.tensor.matmul(wps[96:128, 0:512], v_s[0][:, 0:32],
                         v_s[0][:, 0:512], start=False, stop=True,
                         tile_position=(0, 96))
    return nc


# ---------------- host-side tables & shard prep -------------------------

def host_tables(T: int = 2048):
    n = HD // 4
    af = (1.0 / 1024) ** np.linspace(0, 1, n, dtype=np.float32)
    af = np.concatenate([af, np.zeros(n, np.float32)])  # [32]
    theta = np.outer(np.arange(T, dtype=np.float32), af)  # [T, 32]
    cosT = np.cos(theta).T.astype(np.float32)  # [32, T]
    sinT = np.sin(theta).T.astype(np.float32)
    c2 = np.tile(cosT, (4, 1))                             # [128, T]
    s2 = np.tile(np.concatenate([sinT, -sinT], 0), (2, 1))  # [128, T]
    c2r = np.ascontiguousarray(
        np.broadcast_to(c2[:, None, :], (128, 4, T))).astype(np.float16)
    s2r = np.ascontiguousarray(
        np.broadcast_to(s2[:, None, :], (128, 4, T))).astype(np.float32)
    km = np.arange(128)
    pswap = (km[:, None] == (km[None, :] ^ 32)).astype(np.float16)
    bdiag = ((km[:, None] // 64) == (km[None, :] // 64)).astype(np.float16)
    r_ = np.arange(128)[:, None]
    c_ = np.arange(128)[None, :]
    tri1 = np.where(c_ >= r_, 1.0, 0.0).astype(np.float16)
    tri = np.concatenate([tri1, tri1], axis=1)             # [128, 256]
    return {"c2": c2r, "s2": s2r,
            "pswap": pswap, "bdiag": bdiag,
            "trimask": np.ascontiguousarray(tri)}


def core_inputs(x, wq, wk, wv, wo, core: int, T: int = 2048):
    b, g = core % 4, core // 4
    sl = slice(g * DH, (g + 1) * DH)
    m = {
        "xt": np.ascontiguousarray(np.asarray(x[b]).T).astype(np.float16),
        "wqt": np.ascontiguousarray(
            np.asarray(wq)[sl, :].T).astype(np.float16),
        "wkt": np.ascontiguousarray(
            np.asarray(wk)[sl, :].T).astype(np.float16),
        "wvt": np.ascontiguousarray(
            np.asarray(wv)[sl, :].T).astype(np.float16),
        "wot": np.ascontiguousarray(
            np.asarray(wo)[:, sl].T).astype(np.float16),
    }
    m.update(host_tables(T))
    return m


_CACHE = {}


def _get_nc(T: int = 2048):
    key = ("nc", T)
    if key not in _CACHE:
        nc = bacc.Bacc("TRN2", target_bir_lowering=False, debug=False)
        build_kernel(nc, T)
        nc.compile()
        _CACHE[key] = nc
    return _CACHE[key]


def kernel(x, wq, wk, wv, wo, mask=None):
    from concourse import bass_utils
    nc = _get_nc(2048)
    in_maps = [core_inputs(x, wq, wk, wv, wo, c) for c in range(8)]
    res = bass_utils.run_bass_kernel_spmd(nc, in_maps, list(range(8)))
    outs = [np.asarray(res.results[c]["yt"]).astype(np.float32)
            for c in range(8)]
    out = np.empty((4, 2048, 1024), np.float32)
    for b in range(4):
        out[b] = (outs[b] + outs[b + 4]).T
    return out
